# revision 38
# baseline (speedup 1.0000x reference)
"""MLA decode kernel for Trainium2, data-parallel over batch across 8 NeuronCores.

Each core handles 4 batches (M = 16 query rows). Key design points vs the
original baseline:
  - kv/pe caches stored in fp8 (e3m4) in BOTH layouts (halves cache DMA);
    weights stay bf16 (fp8 weights measurably break the 2e-2 error budget).
  - Attention matmuls put the *cache* in the stationary operand so the PE
    array runs 128-wide output partitions:
       scores:  S^T[t,hs]  = kvT_tile[c,t].T @ QT[c,hs]
       PV:      o^T[c,hs] += kvnat_tile[t,c].T @ P^T[t,hs]
    P^T comes straight out of the softmax in the right layout; no P transposes.
  - Softmax without max-subtraction (logits*scale are within +-7 for this
    model; exp in f32 PSUM is safe), fused exp via ScalarE activation, row
    sums via a ones-vector matmul, 1/sum applied to the o^T tile via a
    PE-broadcast outer product.
  - Projections are weight-stationary (activations are only 16 wide), with
    rms_norm done in the transposed layout (partition-dim reduction via
    ones-matmul). Rope is done in transposed layout with even/odd rows
    pre-split (host permutes wq_b/wkv_a rope columns and pe_cache rows).
Host prep does layout/dtype only (transposes, tiling, fp8 cast) - no math.
"""

import os
import sys

sys.path.insert(0, "/opt/trn_rl_repo")

import numpy as np
import ml_dtypes

import concourse.bass as bass
import concourse.bacc as bacc_mod
import concourse.mybir as mybir
from concourse.bass_utils import run_bass_kernel_spmd
from concourse.masks import make_identity
from concourse.tile import TileContext

BF16 = mybir.dt.bfloat16
F32 = mybir.dt.float32
E3 = mybir.dt.float8e3
NBF = ml_dtypes.bfloat16
NE3 = ml_dtypes.float8_e3m4

DIM = 2048
N_HEADS = 16
Q_LORA = 1536
KV_LORA = 512
QK_NOPE = 128
QK_ROPE = 64
V_DIM = 128
QK_HD = QK_NOPE + QK_ROPE  # 192
MAX_SEQ = 8192
BSZ = 32
SEQLEN = 4
START_POS = MAX_SEQ - SEQLEN
EPS = 1e-6
SCALE = QK_HD ** -0.5

N_CORES = 8
BPC = BSZ // N_CORES          # batches per core = 4
M = BPC * SEQLEN              # rows per core = 16 (b, s)
NTG = 1024                    # t-group size for scores stream
G8 = MAX_SEQ // NTG           # 8 groups per batch
KQ = DIM // 128               # 16 k-chunks of x
KB = Q_LORA // 128            # 12 k-chunks of q_lora
R2 = QK_ROPE // 2             # 32

AF = mybir.ActivationFunctionType
ALU = mybir.AluOpType
AX = mybir.AxisListType


def build_bass(debug=False):
    nc = bacc_mod.Bacc(target_bir_lowering=False)

    # ---- DRAM inputs (per core) ----
    xT = nc.dram_tensor("xT", [128, KQ, M], BF16, kind="ExternalInput")
    wqa = nc.dram_tensor("wqa", [KQ, 128, KB, 128], BF16, kind="ExternalInput")
    wqbn = nc.dram_tensor("wqbn", [KB, 128, N_HEADS, 128], BF16, kind="ExternalInput")
    wqbp = nc.dram_tensor("wqbp", [KB, 128, N_HEADS, QK_ROPE], BF16, kind="ExternalInput")
    wkval = nc.dram_tensor("wkval", [128, KQ, 4, 128], BF16, kind="ExternalInput")
    wkvap = nc.dram_tensor("wkvap", [128, KQ, QK_ROPE], BF16, kind="ExternalInput")
    wkvbn = nc.dram_tensor("wkvbn", [128, N_HEADS, KV_LORA], BF16, kind="ExternalInput")
    wkvbv = nc.dram_tensor("wkvbv", [128, N_HEADS, 4, V_DIM], BF16, kind="ExternalInput")
    wo = nc.dram_tensor("wo", [4, 4, 128, 4, 512], BF16, kind="ExternalInput")
    qnwT = nc.dram_tensor("qnwT", [128, KB, 1], F32, kind="ExternalInput")
    kvnwT = nc.dram_tensor("kvnwT", [128, 4, 1], F32, kind="ExternalInput")
    cosq = nc.dram_tensor("cosq", [R2, N_HEADS, M], F32, kind="ExternalInput")
    sinq = nc.dram_tensor("sinq", [R2, N_HEADS, M], F32, kind="ExternalInput")
    cosk = nc.dram_tensor("cosk", [R2, M], F32, kind="ExternalInput")
    sink = nc.dram_tensor("sink", [R2, M], F32, kind="ExternalInput")
    klatS = nc.dram_tensor("klatS", [BPC, G8, 128, 4, NTG], E3, kind="ExternalInput")
    peS = nc.dram_tensor("peS", [BPC, G8, QK_ROPE, NTG], E3, kind="ExternalInput")
    kvnP = nc.dram_tensor("kvnP", [BPC, 4, 128, 16, 512], E3, kind="ExternalInput")
    out = nc.dram_tensor("out", [M, DIM], F32, kind="ExternalOutput")
    if debug:
        dq1nT = nc.dram_tensor("dq1nT", [128, KB, M], F32, kind="ExternalOutput")
        dkvlatT = nc.dram_tensor("dkvlatT", [128, 4, M], F32, kind="ExternalOutput")
        dkpeT = nc.dram_tensor("dkpeT", [QK_ROPE, M], F32, kind="ExternalOutput")
        dQT = nc.dram_tensor("dQT", [128, 5, BPC, 64], F32, kind="ExternalOutput")
        dssum = nc.dram_tensor("dssum", [BPC, 1, 512], F32, kind="ExternalOutput")
        dred = nc.dram_tensor("dred", [BPC, 1, 64], F32, kind="ExternalOutput")
        doutT = nc.dram_tensor("doutT", [128, 4, N_HEADS, M], F32, kind="ExternalOutput")
        do2T = nc.dram_tensor("do2T", [128, N_HEADS, M], F32, kind="ExternalOutput")

    with TileContext(nc) as tc:
        with (
            tc.tile_pool(name="const", bufs=1) as cpool,
            tc.tile_pool(name="acts", bufs=1) as apool,
            tc.tile_pool(name="wqa_s", bufs=4) as wqa_pool,
            tc.tile_pool(name="wqb_s", bufs=3) as wqb_pool,
            tc.tile_pool(name="wqbp_s", bufs=2) as wqbp_pool,
            tc.tile_pool(name="wo_s", bufs=4) as wo_pool,
            tc.tile_pool(name="kl_s", bufs=4) as kl_pool,
            tc.tile_pool(name="pe_s", bufs=4) as pe_pool,
            tc.tile_pool(name="kv_s", bufs=4) as kv_pool,
            tc.tile_pool(name="pt_s", bufs=10) as pt_pool,
            tc.tile_pool(name="ps_small", bufs=3, space="PSUM") as ps_small,
            tc.tile_pool(name="ps_phk", bufs=2, space="PSUM") as ps_phk,
            tc.tile_pool(name="ps_sc", bufs=2, space="PSUM") as ps_sc,
            tc.tile_pool(name="ps_pv", bufs=1, space="PSUM") as ps_pv,
        ):
            # ---------- constants / small residents ----------
            ident = cpool.tile([128, 128], BF16)
            make_identity(nc, ident)
            ones_cb = cpool.tile([128, 1], BF16)
            nc.gpsimd.memset(ones_cb, 1.0)
            ones_cf = cpool.tile([128, 1], F32)
            nc.gpsimd.memset(ones_cf, 1.0)
            ones_rf = cpool.tile([1, 128], F32)
            nc.gpsimd.memset(ones_rf, 1.0)
            eps1 = cpool.tile([1, 1], F32)
            nc.gpsimd.memset(eps1, EPS)

            xT_sb = cpool.tile([128, KQ, M], BF16)
            nc.sync.dma_start(xT_sb, xT[:, :, :])
            qnwT_sb = cpool.tile([128, KB, 1], F32)
            nc.sync.dma_start(qnwT_sb, qnwT[:, :, :])
            kvnwT_sb = cpool.tile([128, 4, 1], F32)
            nc.sync.dma_start(kvnwT_sb, kvnwT[:, :, :])
            cosq_sb = cpool.tile([R2, N_HEADS, M], F32)
            nc.sync.dma_start(cosq_sb, cosq[:, :, :])
            sinq_sb = cpool.tile([R2, N_HEADS, M], F32)
            nc.sync.dma_start(sinq_sb, sinq[:, :, :])
            cosk_sb = cpool.tile([R2, M], F32)
            nc.sync.dma_start(cosk_sb, cosk[:, :])
            sink_sb = cpool.tile([R2, M], F32)
            nc.sync.dma_start(sink_sb, sink[:, :])

            # resident weights (gpsimd queue)
            wkval_sb = cpool.tile([128, KQ, 4, 128], BF16)
            nc.gpsimd.dma_start(wkval_sb, wkval[:, :, :, :])
            wkvap_sb = cpool.tile([128, KQ, QK_ROPE], BF16)
            nc.gpsimd.dma_start(wkvap_sb, wkvap[:, :, :])
            wkvbn_sb = cpool.tile([128, N_HEADS, KV_LORA], BF16)
            nc.gpsimd.dma_start(wkvbn_sb, wkvbn[:, :, :])
            wkvbv_sb = cpool.tile([128, N_HEADS, 4, V_DIM], BF16)
            nc.gpsimd.dma_start(wkvbv_sb, wkvbv[:, :, :, :])

            def rms_factor(ps_in, nchunks, nfeat):
                """Per-column 1/rms broadcast [128, M] f32 from transposed psum."""
                sq = apool.tile([128, nchunks, M], F32, tag="rms_sq")
                nc.scalar.activation(out=sq, in_=ps_in, func=AF.Square)
                msq = ps_small.tile([1, nchunks * M], F32, tag="ps")
                nc.tensor.matmul(msq, ones_cf,
                                 sq.rearrange("p a m -> p (a m)"),
                                 start=True, stop=True)
                # sum the nchunks partials with explicit adds (no aliasing)
                msq_sb = apool.tile([1, nchunks * M], F32, tag="rms_msq")
                nc.vector.tensor_copy(msq_sb, msq)
                msq_v = msq_sb.rearrange("p (a m) -> p a m", a=nchunks)
                prev = msq_v[:, 0, :]
                for a in range(1, nchunks):
                    red = apool.tile([1, M], F32, tag=f"rms_red{a % 2}")
                    nc.vector.tensor_tensor(red, prev, msq_v[:, a, :], op=ALU.add)
                    prev = red
                rstd = apool.tile([1, M], F32, tag="rms_rstd")
                nc.scalar.activation(
                    out=rstd, in_=red,
                    func=AF.Sqrt, scale=1.0 / nfeat, bias=eps1)
                rinv = apool.tile([1, M], F32, tag="rms_rinv")
                nc.vector.reciprocal(rinv, rstd)
                bc_ps = ps_small.tile([128, M], F32, tag="ps")
                nc.tensor.matmul(bc_ps, ones_rf, rinv, start=True, stop=True)
                bc = apool.tile([128, M], F32, tag="rms_bc")
                nc.vector.tensor_copy(bc, bc_ps)
                return bc

            # ---------- kv path: kvT = (x @ wkv_a)^T, transposed layout ----------
            # j-outer: one open accumulation group per PSUM bank at a time
            pskv = ps_small.tile([128, 4, M], F32, tag="ps")
            for j in range(4):
                for k in range(KQ):
                    nc.tensor.matmul(pskv[:, j, :], wkval_sb[:, k, j, :],
                                     xT_sb[:, k, :],
                                     start=(k == 0), stop=(k == KQ - 1))

            bckv = rms_factor(pskv, 4, KV_LORA)
            kvlatT_bf = apool.tile([128, 4, M], BF16)
            for j in range(4):
                nc.vector.scalar_tensor_tensor(
                    out=kvlatT_bf[:, j, :], in0=pskv[:, j, :],
                    scalar=kvnwT_sb[:, j, :], in1=bckv,
                    op0=ALU.mult, op1=ALU.mult)
            kvlatT8 = apool.tile([128, 4, M], E3)
            nc.vector.tensor_copy(kvlatT8, kvlatT_bf)
            # natural-layout fp8 copy for the PV-side insert
            kvlat8 = apool.tile([M, KV_LORA], E3)
            for j in range(4):
                ptb = ps_small.tile([M, 128], BF16, tag="ps")
                nc.tensor.transpose(ptb, kvlatT_bf[:, j, :], ident)
                nc.vector.tensor_copy(kvlat8[:, j * 128:(j + 1) * 128], ptb)

            # k_pe projection + rope in transposed even/odd-split layout
            pskp = ps_small.tile([QK_ROPE, M], F32, tag="ps")
            for k in range(KQ):
                nc.tensor.matmul(pskp, wkvap_sb[:, k, :], xT_sb[:, k, :],
                                 start=(k == 0), stop=(k == KQ - 1))
            kpeT8 = apool.tile([QK_ROPE, M], E3)
            t1k = apool.tile([R2, M], F32, tag="ropek1")
            t2k = apool.tile([R2, M], F32, tag="ropek2")
            nc.vector.tensor_tensor(t1k, pskp[0:R2, :], cosk_sb, op=ALU.mult)
            nc.vector.tensor_tensor(t2k, pskp[R2:QK_ROPE, :], sink_sb, op=ALU.mult)
            nc.vector.tensor_tensor(kpeT8[0:R2, :], t1k, t2k, op=ALU.subtract)
            t1k2 = apool.tile([R2, M], F32, tag="ropek1")
            t2k2 = apool.tile([R2, M], F32, tag="ropek2")
            nc.vector.tensor_tensor(t1k2, pskp[0:R2, :], sink_sb, op=ALU.mult)
            nc.vector.tensor_tensor(t2k2, pskp[R2:QK_ROPE, :], cosk_sb, op=ALU.mult)
            nc.vector.tensor_tensor(kpeT8[R2:QK_ROPE, :], t1k2, t2k2, op=ALU.add)

            # ---------- q path ----------
            # streamed weights force k-outer loops; PSUM cannot hold multiple
            # interleaved accumulation groups per bank, so each (k, chunk)
            # product is a closed single-matmul group, accumulated on DVE.
            def acc_chunk(acc_slice, phk, first):
                if first:
                    nc.vector.tensor_copy(acc_slice, phk)
                else:
                    nc.vector.tensor_tensor(acc_slice, acc_slice, phk, op=ALU.add)

            q1T_acc = apool.tile([128, KB, M], F32)
            for k in range(KQ):
                wqa_sb = wqa_pool.tile([128, KB, 128], BF16, tag="wqa")
                nc.scalar.dma_start(wqa_sb, wqa[k, :, :, :])
                for ng in range(KB // 4):
                    phk = ps_phk.tile([128, 4, M], F32, tag="phk")
                    for j in range(4):
                        nc.tensor.matmul(phk[:, j, :], wqa_sb[:, ng * 4 + j, :],
                                         xT_sb[:, k, :], start=True, stop=True)
                    acc_chunk(q1T_acc[:, ng * 4:(ng + 1) * 4, :], phk, k == 0)
            bcq = rms_factor(q1T_acc, KB, Q_LORA)
            q1nT = apool.tile([128, KB, M], BF16)
            for kc in range(KB):
                nc.vector.scalar_tensor_tensor(
                    out=q1nT[:, kc, :], in0=q1T_acc[:, kc, :],
                    scalar=qnwT_sb[:, kc, :], in1=bcq,
                    op0=ALU.mult, op1=ALU.mult)

            qnT_acc = apool.tile([128, N_HEADS, M], F32)
            # rope-dim accumulators split at base partition 0 (SB-SB tensor
            # ops require equal base partitions on both inputs)
            qpe_e = apool.tile([R2, N_HEADS, M], F32)
            qpe_o = apool.tile([R2, N_HEADS, M], F32)
            for k in range(KB):
                wqbn_sb = wqb_pool.tile([128, N_HEADS, 128], BF16, tag="wqbn")
                nc.sync.dma_start(wqbn_sb, wqbn[k, :, :, :])
                wqbp_sb = wqbp_pool.tile([128, N_HEADS, QK_ROPE], BF16, tag="wqbp")
                nc.sync.dma_start(wqbp_sb, wqbp[k, :, :, :])
                for hg in range(N_HEADS // 4):
                    phk = ps_phk.tile([128, 4, M], F32, tag="phk")
                    for j in range(4):
                        nc.tensor.matmul(phk[:, j, :],
                                         wqbn_sb[:, hg * 4 + j, :],
                                         q1nT[:, k, :], start=True, stop=True)
                    acc_chunk(qnT_acc[:, hg * 4:(hg + 1) * 4, :], phk, k == 0)
                for hg in range(N_HEADS // 4):
                    php = ps_phk.tile([QK_ROPE, 4, M], F32, tag="phk")
                    for j in range(4):
                        nc.tensor.matmul(php[:, j, :],
                                         wqbp_sb[:, hg * 4 + j, :],
                                         q1nT[:, k, :], start=True, stop=True)
                    acc_chunk(qpe_e[:, hg * 4:(hg + 1) * 4, :],
                              php[0:R2, :, :], k == 0)
                    acc_chunk(qpe_o[:, hg * 4:(hg + 1) * 4, :],
                              php[R2:QK_ROPE, :, :], k == 0)

            if debug:
                def dump(dram_ap, src_ap, shape):
                    t = apool.tile(shape, F32, tag="dbg")
                    nc.vector.tensor_copy(t, src_ap)
                    nc.sync.dma_start(dram_ap, t)
                dump(dq1nT[:, :, :], q1nT, [128, KB, M])
                dump(dkvlatT[:, :, :], kvlatT_bf, [128, 4, M])
                dump(dkpeT[:, :], kpeT8, [QK_ROPE, M])

            qnT_sb = apool.tile([128, N_HEADS, M], BF16)
            nc.vector.tensor_copy(qnT_sb, qnT_acc)

            # QT[:, 0:4, b, hs] = absorbed nope; QT[:64, 4, b, hs] = roped pe
            QT = apool.tile([128, 5, BPC, 64], BF16)

            # q_pe rope (all heads, even/odd-split rows)
            t1q = apool.tile([R2, N_HEADS, M], F32, tag="ropeq1")
            t2q = apool.tile([R2, N_HEADS, M], F32, tag="ropeq2")
            nc.vector.tensor_tensor(t1q, qpe_e, cosq_sb, op=ALU.mult)
            nc.vector.tensor_tensor(t2q, qpe_o, sinq_sb, op=ALU.mult)
            for b in range(BPC):
                nc.vector.tensor_tensor(
                    QT[0:R2, 4, b, :].rearrange("p (h s) -> p h s", h=N_HEADS),
                    t1q[:, :, b * SEQLEN:(b + 1) * SEQLEN],
                    t2q[:, :, b * SEQLEN:(b + 1) * SEQLEN],
                    op=ALU.subtract)
            t3q = apool.tile([R2, N_HEADS, M], F32, tag="ropeq1")
            t4q = apool.tile([R2, N_HEADS, M], F32, tag="ropeq2")
            nc.vector.tensor_tensor(t3q, qpe_e, sinq_sb, op=ALU.mult)
            nc.vector.tensor_tensor(t4q, qpe_o, cosq_sb, op=ALU.mult)
            for b in range(BPC):
                nc.vector.tensor_tensor(
                    QT[R2:QK_ROPE, 4, b, :].rearrange("p (h s) -> p h s", h=N_HEADS),
                    t3q[:, :, b * SEQLEN:(b + 1) * SEQLEN],
                    t4q[:, :, b * SEQLEN:(b + 1) * SEQLEN],
                    op=ALU.add)

            # absorb: QT[:, cc, b, h*4+s] = sum_d wkvbn[h][d, c] * qnope[m, h, d]
            for h in range(N_HEADS):
                pa4 = ps_small.tile([128, 4, M], F32, tag="ps")
                for cc in range(4):
                    nc.tensor.matmul(pa4[:, cc, :],
                                     wkvbn_sb[:, h, cc * 128:(cc + 1) * 128],
                                     qnT_sb[:, h, :], start=True, stop=True)
                for cc in range(4):
                    nc.vector.tensor_copy(
                        QT[:, cc, :, h * SEQLEN:(h + 1) * SEQLEN],
                        pa4[:, cc, :].rearrange("p (b s) -> p b s", b=BPC))

            if debug:
                dump(dQT[:, :, :, :], QT, [128, 5, BPC, 64])

            # ---------- attention ----------
            outT = apool.tile([128, 4, N_HEADS, M], BF16)
            for b in range(BPC):
                po = ps_pv.tile([128, 4, 64], F32, tag="po")
                PTs = []
                kvs = []
                ssum_prev = None
                for g in range(G8):
                    kl = kl_pool.tile([128, 4, NTG], E3, tag="kl")
                    nc.sync.dma_start(kl, klatS[b, g])
                    pe_t = pe_pool.tile([QK_ROPE, NTG], E3, tag="pe")
                    nc.sync.dma_start(pe_t, peS[b, g])
                    if g % 2 == 0:
                        kv16 = kv_pool.tile([128, 16, 512], E3, tag="kv16")
                        nc.scalar.dma_start(kv16, kvnP[b, g // 2])
                        kvs.append(kv16)
                    if g == G8 - 1:
                        # overwrite rows start_pos..start_pos+3 with fresh values
                        for j in range(4):
                            nc.sync.dma_start(
                                kl[:, j, NTG - SEQLEN:],
                                kvlatT8[:, j, b * SEQLEN:(b + 1) * SEQLEN])
                        nc.sync.dma_start(
                            pe_t[:, NTG - SEQLEN:],
                            kpeT8[:, b * SEQLEN:(b + 1) * SEQLEN])
                        nc.sync.dma_start(
                            kvs[-1][128 - SEQLEN:, 15, :],
                            kvlat8[b * SEQLEN:(b + 1) * SEQLEN, :])
                    sp = ps_sc.tile([128, 8, 64], F32, tag="sp")
                    for i in range(8):
                        for j in range(4):
                            nc.tensor.matmul(sp[:, i, :],
                                             kl[:, j, i * 128:(i + 1) * 128],
                                             QT[:, j, b, :],
                                             start=(j == 0), stop=False)
                        nc.tensor.matmul(sp[:, i, :],
                                         pe_t[:, i * 128:(i + 1) * 128],
                                         QT[0:QK_ROPE, 4, b, :],
                                         start=False, stop=True)
                    PT = pt_pool.tile([128, 8, 64], BF16, tag="PT")
                    nc.scalar.activation(out=PT, in_=sp, func=AF.Exp, scale=SCALE)
                    PTs.append(PT)
                    # partition row-sums on GpSimd (PE-free), accumulate on DVE
                    gp = apool.tile([1, 8, 64], F32, tag=f"gp{g % 2}")
                    nc.gpsimd.tensor_reduce(gp, PT, axis=AX.C, op=ALU.add)
                    ssum_g = apool.tile([1, 512], F32, tag=f"att_s{g % 2}")
                    if ssum_prev is None:
                        nc.vector.tensor_copy(
                            ssum_g, gp.rearrange("p a h -> p (a h)"))
                    else:
                        nc.vector.tensor_tensor(
                            ssum_g, ssum_prev, gp.rearrange("p a h -> p (a h)"),
                            op=ALU.add)
                    ssum_prev = ssum_g

                # PV: cc-outer so each PSUM-bank accumulation group is
                # open exclusively (interleaved groups in one bank corrupt)
                for cc in range(4):
                    for g in range(G8):
                        for i in range(8):
                            nc.tensor.matmul(
                                po[:, cc, :],
                                kvs[g // 2][:, (g % 2) * 8 + i,
                                            cc * 128:(cc + 1) * 128],
                                PTs[g][:, i, :],
                                start=(g == 0 and i == 0),
                                stop=(g == G8 - 1 and i == 7))

                # 1/rowsum: tree-reduce the per-chunk partials [1,(i,h)]->[1,h]
                s4 = apool.tile([1, 256], F32, tag="att_r4")
                nc.vector.tensor_tensor(s4, ssum_prev[:, 0:256],
                                        ssum_prev[:, 256:512], op=ALU.add)
                s2 = apool.tile([1, 128], F32, tag="att_r2")
                nc.vector.tensor_tensor(s2, s4[:, 0:128], s4[:, 128:256],
                                        op=ALU.add)
                red = apool.tile([1, 64], F32, tag="att_red")
                nc.vector.tensor_tensor(red, s2[:, 0:64], s2[:, 64:128],
                                        op=ALU.add)
                rinv = apool.tile([1, 64], F32, tag="att_rinv")
                nc.vector.reciprocal(rinv, red)
                if debug:
                    dump(dssum[b], ssum_prev, [1, 512])
                    dump(dred[b], red, [1, 64])
                bc_ps = ps_small.tile([128, 64], F32, tag="ps")
                nc.tensor.matmul(bc_ps, ones_rf, rinv, start=True, stop=True)
                bc = apool.tile([128, 64], F32, tag="att_bc")
                nc.vector.tensor_copy(bc, bc_ps)
                for cc in range(4):
                    nc.vector.tensor_tensor(
                        outT[:, cc, :, b * SEQLEN:(b + 1) * SEQLEN],
                        po[:, cc, :].rearrange("p (h s) -> p h s", h=N_HEADS),
                        bc.rearrange("p (h s) -> p h s", h=N_HEADS),
                        op=ALU.mult)

            if debug:
                dump(doutT[:, :, :, :], outT, [128, 4, N_HEADS, M])

            # ---------- v-proj: o2T[d, h, m] ----------
            o2T = apool.tile([128, N_HEADS, M], BF16)
            for h in range(N_HEADS):
                pv = ps_small.tile([128, M], F32, tag="ps")
                for cc in range(4):
                    nc.tensor.matmul(pv, wkvbv_sb[:, h, cc, :],
                                     outT[:, cc, h, :],
                                     start=(cc == 0), stop=(cc == 3))
                nc.vector.tensor_copy(o2T[:, h, :], pv)

            if debug:
                dump(do2T[:, :, :], o2T, [128, N_HEADS, M])

            # ---------- final: out = o2 @ wo ----------
            for n in range(4):
                pf = ps_small.tile([M, 512], F32, tag="ps")
                for kk in range(4):
                    wot = wo_pool.tile([128, 4, 512], BF16, tag="wo")
                    nc.gpsimd.dma_start(wot, wo[n, kk])
                    for j in range(4):
                        k = kk * 4 + j
                        nc.tensor.matmul(pf, o2T[:, k, :], wot[:, j, :],
                                         start=(k == 0), stop=(k == KQ - 1))
                fin = pt_pool.tile([M, 512], F32, tag="fin")
                nc.vector.tensor_copy(fin, pf)
                nc.sync.dma_start(out[:, n * 512:(n + 1) * 512], fin)

    nc.compile()
    return nc


_NC_CACHE = {}

# even/odd split permutation for rope dims: rows 0..31 = even pairs, 32..63 = odd
_PERM = np.concatenate([np.arange(0, QK_ROPE, 2), np.arange(1, QK_ROPE, 2)])


def prep_in_maps(x, wq_a, q_norm_w, wq_b, wkv_a, kv_norm_w, wkv_b, wo,
                 kv_cache, pe_cache, freqs_cos, freqs_sin, start_pos):
    assert int(start_pos) == START_POS
    bf = lambda a: np.ascontiguousarray(np.asarray(a, dtype=np.float32), dtype=NBF)
    f32 = lambda a: np.ascontiguousarray(np.asarray(a), dtype=np.float32)
    c = np.ascontiguousarray

    x = f32(x)
    wq_a = f32(wq_a); wq_b = f32(wq_b); wkv_a = f32(wkv_a)
    wkv_b_r = f32(wkv_b).reshape(N_HEADS, QK_NOPE + V_DIM, KV_LORA)
    wo_f = f32(wo)

    # --- weights, transposed/tiled layouts (bf16) ---
    wqa_t = bf(wq_a.reshape(KQ, 128, KB, 128))                    # [k,p,nc,d]
    wqb_r = wq_b.reshape(KB, 128, N_HEADS, QK_HD)                 # [k,p,h,d]
    wqbn_t = bf(wqb_r[:, :, :, :QK_NOPE])
    wqbp_t = bf(wqb_r[:, :, :, QK_NOPE:][:, :, :, _PERM])         # rope perm
    wkva_r = wkv_a.reshape(KQ, 128, KV_LORA + QK_ROPE)
    wkval_t = bf(wkva_r[:, :, :KV_LORA].reshape(KQ, 128, 4, 128)
                 .transpose(1, 0, 2, 3))                          # [p,k,j,d]
    wkvap_t = bf(wkva_r[:, :, KV_LORA:][:, :, _PERM].transpose(1, 0, 2))
    wkvbn_t = bf(wkv_b_r[:, :QK_NOPE, :].transpose(1, 0, 2))      # [d,h,c]
    # [c-in-chunk, h, cc, d]
    wkvbv_t = bf(wkv_b_r[:, QK_NOPE:, :].transpose(2, 0, 1)
                 .reshape(4, 128, N_HEADS, V_DIM).transpose(1, 2, 0, 3))
    # [n, kk, p, j, cc]: wo_t[n,kk,p,j,cc] = wo[(kk*4+j)*128+p, n*512+cc]
    wo_t = bf(wo_f.reshape(4, 4, 128, 4, 512).transpose(3, 0, 2, 1, 4))

    qnwT = c(f32(q_norm_w).reshape(KB, 128, 1).transpose(1, 0, 2))
    kvnwT = c(f32(kv_norm_w).reshape(4, 128, 1).transpose(1, 0, 2))

    # --- rope tables, transposed to [pair, (h), m], even/odd split is implicit
    # (row r < 32 uses cos[r], row 32+r also uses cos[r]; table holds cos[i,m])
    cos = f32(freqs_cos); sin = f32(freqs_sin)                    # [s=4, 32]
    cosT = np.tile(cos.T, (1, BPC))                               # [32, 16] m=(b,s)
    sinT = np.tile(sin.T, (1, BPC))
    cosq_t = c(np.repeat(cosT[:, None, :], N_HEADS, axis=1))      # [32, h, 16]
    sinq_t = c(np.repeat(sinT[:, None, :], N_HEADS, axis=1))

    # --- fp8 caches in both layouts ---
    kv8 = np.asarray(kv_cache, dtype=np.float32).astype(NE3)      # [32, 8192, 512]
    pe8 = np.asarray(pe_cache, dtype=np.float32).astype(NE3)      # [32, 8192, 64]

    in_maps = []
    for ci in range(N_CORES):
        bs = slice(ci * BPC, (ci + 1) * BPC)
        kvb = kv8[bs]                                             # [4, 8192, 512]
        peb = pe8[bs]
        # scores-side: klatS[b,g,p,j,n] = kv[b, g*1024+n, j*128+p]
        klatS = c(kvb.reshape(BPC, G8, NTG, 4, 128).transpose(0, 1, 4, 3, 2))
        # peS[b,g,r,n] = pe[b, g*1024+n, perm(r)]
        peS_ = c(peb[:, :, _PERM].reshape(BPC, G8, NTG, QK_ROPE)
                 .transpose(0, 1, 3, 2))
        # pv-side: kvnP[b,gg,p,i,c] = kv[b, gg*2048 + i*128 + p, c]
        kvnP_ = c(kvb.reshape(BPC, 4, 16, 128, KV_LORA).transpose(0, 1, 3, 2, 4))
        xc = bf(x[bs].reshape(M, DIM).T.reshape(KQ, 128, M).transpose(1, 0, 2))
        in_maps.append({
            "xT": xc,
            "wqa": wqa_t, "wqbn": wqbn_t, "wqbp": wqbp_t,
            "wkval": wkval_t, "wkvap": wkvap_t,
            "wkvbn": wkvbn_t, "wkvbv": wkvbv_t, "wo": wo_t,
            "qnwT": qnwT, "kvnwT": kvnwT,
            "cosq": cosq_t, "sinq": sinq_t, "cosk": c(cosT), "sink": c(sinT),
            "klatS": klatS, "peS": peS_, "kvnP": kvnP_,
        })
    return in_maps


def kernel(**inputs):
    in_maps = prep_in_maps(**inputs)

    debug = os.environ.get("KERNEL_DEBUG", "0") == "1"
    if "nc" not in _NC_CACHE:
        _NC_CACHE["nc"] = build_bass(debug=debug)
    nc = _NC_CACHE["nc"]

    trace = os.environ.get("KERNEL_TRACE", "0") == "1"
    res = run_bass_kernel_spmd(nc, in_maps, core_ids=list(range(N_CORES)), trace=trace)
    if trace and res.exec_time_ns is not None:
        print(f"HW exec time: {res.exec_time_ns} ns")
        _NC_CACHE["last_exec_ns"] = res.exec_time_ns

    _NC_CACHE["results"] = res.results
    outs = [r["out"].reshape(BPC, SEQLEN, DIM) for r in res.results]
    return np.concatenate(outs, axis=0).astype(np.float32)


# revision 43
# speedup vs baseline: 6.4114x; 6.4114x over previous
"""MLA decode kernel for Trainium2, data-parallel over batch across 8 NeuronCores.

Each core handles 4 batches (M = 16 query rows). Key design points vs the
original baseline:
  - kv/pe caches stored in fp8 (e3m4) in BOTH layouts (halves cache DMA);
    weights stay bf16 (fp8 weights measurably break the 2e-2 error budget).
  - Attention matmuls put the *cache* in the stationary operand so the PE
    array runs 128-wide output partitions:
       scores:  S^T[t,hs]  = kvT_tile[c,t].T @ QT[c,hs]
       PV:      o^T[c,hs] += kvnat_tile[t,c].T @ P^T[t,hs]
    P^T comes straight out of the softmax in the right layout; no P transposes.
  - Softmax without max-subtraction (logits*scale are within +-7 for this
    model; exp in f32 PSUM is safe), fused exp via ScalarE activation, row
    sums via a ones-vector matmul, 1/sum applied to the o^T tile via a
    PE-broadcast outer product.
  - Projections are weight-stationary (activations are only 16 wide), with
    rms_norm done in the transposed layout (partition-dim reduction via
    ones-matmul). Rope is done in transposed layout with even/odd rows
    pre-split (host permutes wq_b/wkv_a rope columns and pe_cache rows).
Host prep does layout/dtype only (transposes, tiling, fp8 cast) - no math.
"""

import os
import sys

sys.path.insert(0, "/opt/trn_rl_repo")

import numpy as np
import ml_dtypes

import concourse.bass as bass
import concourse.bacc as bacc_mod
import concourse.mybir as mybir
from concourse.bass_utils import run_bass_kernel_spmd
from concourse.masks import make_identity
from concourse.tile import TileContext

BF16 = mybir.dt.bfloat16
F32 = mybir.dt.float32
E3 = mybir.dt.float8e3
NBF = ml_dtypes.bfloat16
NE3 = ml_dtypes.float8_e3m4

DIM = 2048
N_HEADS = 16
Q_LORA = 1536
KV_LORA = 512
QK_NOPE = 128
QK_ROPE = 64
V_DIM = 128
QK_HD = QK_NOPE + QK_ROPE  # 192
MAX_SEQ = 8192
BSZ = 32
SEQLEN = 4
START_POS = MAX_SEQ - SEQLEN
EPS = 1e-6
SCALE = QK_HD ** -0.5

N_CORES = 8
BPC = BSZ // N_CORES          # batches per core = 4
M = BPC * SEQLEN              # rows per core = 16 (b, s)
NTG = 1024                    # t-group size for scores stream
G8 = MAX_SEQ // NTG           # 8 groups per batch
KQ = DIM // 128               # 16 k-chunks of x
KB = Q_LORA // 128            # 12 k-chunks of q_lora
R2 = QK_ROPE // 2             # 32

AF = mybir.ActivationFunctionType
ALU = mybir.AluOpType
AX = mybir.AxisListType


def build_bass(debug=False):
    nc = bacc_mod.Bacc(target_bir_lowering=False)

    # ---- DRAM inputs (per core) ----
    xT = nc.dram_tensor("xT", [128, KQ, M], BF16, kind="ExternalInput")
    wqa = nc.dram_tensor("wqa", [KQ, 128, KB, 128], BF16, kind="ExternalInput")
    wqbn = nc.dram_tensor("wqbn", [KB, 128, N_HEADS, 128], BF16, kind="ExternalInput")
    wqbp = nc.dram_tensor("wqbp", [KB, 128, N_HEADS, QK_ROPE], BF16, kind="ExternalInput")
    wkval = nc.dram_tensor("wkval", [128, KQ, 4, 128], BF16, kind="ExternalInput")
    wkvap = nc.dram_tensor("wkvap", [128, KQ, QK_ROPE], BF16, kind="ExternalInput")
    wkvbn = nc.dram_tensor("wkvbn", [128, N_HEADS, KV_LORA], BF16, kind="ExternalInput")
    wkvbv = nc.dram_tensor("wkvbv", [128, N_HEADS, 4, V_DIM], BF16, kind="ExternalInput")
    wo = nc.dram_tensor("wo", [4, 4, 128, 4, 512], BF16, kind="ExternalInput")
    qnwT = nc.dram_tensor("qnwT", [128, KB, 1], F32, kind="ExternalInput")
    kvnwT = nc.dram_tensor("kvnwT", [128, 4, 1], F32, kind="ExternalInput")
    cosq = nc.dram_tensor("cosq", [R2, N_HEADS, M], F32, kind="ExternalInput")
    sinq = nc.dram_tensor("sinq", [R2, N_HEADS, M], F32, kind="ExternalInput")
    cosk = nc.dram_tensor("cosk", [R2, M], F32, kind="ExternalInput")
    sink = nc.dram_tensor("sink", [R2, M], F32, kind="ExternalInput")
    klatS = nc.dram_tensor("klatS", [BPC, G8, 128, 4, NTG], E3, kind="ExternalInput")
    peS = nc.dram_tensor("peS", [BPC, G8, QK_ROPE, NTG], E3, kind="ExternalInput")
    kvnP = nc.dram_tensor("kvnP", [BPC, 4, 128, 16, 512], E3, kind="ExternalInput")
    out = nc.dram_tensor("out", [M, DIM], F32, kind="ExternalOutput")
    if debug:
        dq1nT = nc.dram_tensor("dq1nT", [128, KB, M], F32, kind="ExternalOutput")
        dkvlatT = nc.dram_tensor("dkvlatT", [128, 4, M], F32, kind="ExternalOutput")
        dkpeT = nc.dram_tensor("dkpeT", [QK_ROPE, M], F32, kind="ExternalOutput")
        dQT = nc.dram_tensor("dQT", [128, 5, BPC, 64], F32, kind="ExternalOutput")
        dssum = nc.dram_tensor("dssum", [BPC, 1, 512], F32, kind="ExternalOutput")
        dred = nc.dram_tensor("dred", [BPC, 1, 64], F32, kind="ExternalOutput")
        doutT = nc.dram_tensor("doutT", [128, 4, N_HEADS, M], F32, kind="ExternalOutput")
        do2T = nc.dram_tensor("do2T", [128, N_HEADS, M], F32, kind="ExternalOutput")

    with TileContext(nc) as tc:
        with (
            tc.tile_pool(name="const", bufs=1) as cpool,
            tc.tile_pool(name="acts", bufs=1) as apool,
            tc.tile_pool(name="wqa_s", bufs=4) as wqa_pool,
            tc.tile_pool(name="wqb_s", bufs=3) as wqb_pool,
            tc.tile_pool(name="wqbp_s", bufs=2) as wqbp_pool,
            tc.tile_pool(name="wo_s", bufs=4) as wo_pool,
            tc.tile_pool(name="kl_s", bufs=4) as kl_pool,
            tc.tile_pool(name="pe_s", bufs=4) as pe_pool,
            tc.tile_pool(name="kv_s", bufs=4) as kv_pool,
            tc.tile_pool(name="pt_s", bufs=10) as pt_pool,
            tc.tile_pool(name="ps_small", bufs=2, space="PSUM") as ps_small,
            tc.tile_pool(name="ps_phk", bufs=2, space="PSUM") as ps_phk,
            tc.tile_pool(name="ps_sc", bufs=2, space="PSUM") as ps_sc,
            tc.tile_pool(name="ps_pv", bufs=1, space="PSUM") as ps_pv,
            tc.tile_pool(name="ps_sum", bufs=1, space="PSUM") as ps_sum,
        ):
            # ---------- constants / small residents ----------
            ident = cpool.tile([128, 128], BF16)
            make_identity(nc, ident)
            ones_cb = cpool.tile([128, 1], BF16)
            nc.gpsimd.memset(ones_cb, 1.0)
            ones_cf = cpool.tile([128, 1], F32)
            nc.gpsimd.memset(ones_cf, 1.0)
            ones_rf = cpool.tile([1, 128], F32)
            nc.gpsimd.memset(ones_rf, 1.0)
            eps1 = cpool.tile([1, 1], F32)
            nc.gpsimd.memset(eps1, EPS)

            xT_sb = cpool.tile([128, KQ, M], BF16)
            nc.sync.dma_start(xT_sb, xT[:, :, :])
            qnwT_sb = cpool.tile([128, KB, 1], F32)
            nc.sync.dma_start(qnwT_sb, qnwT[:, :, :])
            kvnwT_sb = cpool.tile([128, 4, 1], F32)
            nc.sync.dma_start(kvnwT_sb, kvnwT[:, :, :])
            cosq_sb = cpool.tile([R2, N_HEADS, M], F32)
            nc.sync.dma_start(cosq_sb, cosq[:, :, :])
            sinq_sb = cpool.tile([R2, N_HEADS, M], F32)
            nc.sync.dma_start(sinq_sb, sinq[:, :, :])
            cosk_sb = cpool.tile([R2, M], F32)
            nc.sync.dma_start(cosk_sb, cosk[:, :])
            sink_sb = cpool.tile([R2, M], F32)
            nc.sync.dma_start(sink_sb, sink[:, :])

            # resident weights (gpsimd queue)
            wkval_sb = cpool.tile([128, KQ, 4, 128], BF16)
            nc.gpsimd.dma_start(wkval_sb, wkval[:, :, :, :])
            wkvap_sb = cpool.tile([128, KQ, QK_ROPE], BF16)
            nc.gpsimd.dma_start(wkvap_sb, wkvap[:, :, :])
            wkvbn_sb = cpool.tile([128, N_HEADS, KV_LORA], BF16)
            nc.gpsimd.dma_start(wkvbn_sb, wkvbn[:, :, :])
            wkvbv_sb = cpool.tile([128, N_HEADS, 4, V_DIM], BF16)
            nc.gpsimd.dma_start(wkvbv_sb, wkvbv[:, :, :, :])

            def rms_factor(ps_in, nchunks, nfeat):
                """Per-column 1/rms broadcast [128, M] f32 from transposed psum."""
                sq = apool.tile([128, nchunks, M], F32, tag="rms_sq")
                nc.scalar.activation(out=sq, in_=ps_in, func=AF.Square)
                msq = ps_small.tile([1, nchunks * M], F32, tag="ps")
                nc.tensor.matmul(msq, ones_cf,
                                 sq.rearrange("p a m -> p (a m)"),
                                 start=True, stop=True)
                # sum the nchunks partials with explicit adds (no aliasing)
                msq_sb = apool.tile([1, nchunks * M], F32, tag="rms_msq")
                nc.vector.tensor_copy(msq_sb, msq)
                msq_v = msq_sb.rearrange("p (a m) -> p a m", a=nchunks)
                prev = msq_v[:, 0, :]
                for a in range(1, nchunks):
                    red = apool.tile([1, M], F32, tag=f"rms_red{a % 2}")
                    nc.vector.tensor_tensor(red, prev, msq_v[:, a, :], op=ALU.add)
                    prev = red
                rstd = apool.tile([1, M], F32, tag="rms_rstd")
                nc.scalar.activation(
                    out=rstd, in_=red,
                    func=AF.Sqrt, scale=1.0 / nfeat, bias=eps1)
                rinv = apool.tile([1, M], F32, tag="rms_rinv")
                nc.vector.reciprocal(rinv, rstd)
                bc_ps = ps_small.tile([128, M], F32, tag="ps")
                nc.tensor.matmul(bc_ps, ones_rf, rinv, start=True, stop=True)
                bc = apool.tile([128, M], F32, tag="rms_bc")
                nc.vector.tensor_copy(bc, bc_ps)
                return bc

            # ---------- kv path: kvT = (x @ wkv_a)^T, transposed layout ----------
            # j-outer: one open accumulation group per PSUM bank at a time
            pskv = ps_small.tile([128, 4, M], F32, tag="ps")
            for j in range(4):
                for k in range(KQ):
                    nc.tensor.matmul(pskv[:, j, :], wkval_sb[:, k, j, :],
                                     xT_sb[:, k, :],
                                     start=(k == 0), stop=(k == KQ - 1))

            bckv = rms_factor(pskv, 4, KV_LORA)
            kvlatT_bf = apool.tile([128, 4, M], BF16)
            for j in range(4):
                nc.vector.scalar_tensor_tensor(
                    out=kvlatT_bf[:, j, :], in0=pskv[:, j, :],
                    scalar=kvnwT_sb[:, j, :], in1=bckv,
                    op0=ALU.mult, op1=ALU.mult)
            kvlatT8 = apool.tile([128, 4, M], E3)
            nc.vector.tensor_copy(kvlatT8, kvlatT_bf)
            # natural-layout fp8 copy for the PV-side insert
            kvlat8 = apool.tile([M, KV_LORA], E3)
            for j in range(4):
                ptb = ps_small.tile([M, 128], BF16, tag="ps")
                nc.tensor.transpose(ptb, kvlatT_bf[:, j, :], ident)
                nc.vector.tensor_copy(kvlat8[:, j * 128:(j + 1) * 128], ptb)

            # k_pe projection + rope in transposed even/odd-split layout
            pskp = ps_small.tile([QK_ROPE, M], F32, tag="ps")
            for k in range(KQ):
                nc.tensor.matmul(pskp, wkvap_sb[:, k, :], xT_sb[:, k, :],
                                 start=(k == 0), stop=(k == KQ - 1))
            kpeT8 = apool.tile([QK_ROPE, M], E3)
            t1k = apool.tile([R2, M], F32, tag="ropek1")
            t2k = apool.tile([R2, M], F32, tag="ropek2")
            nc.vector.tensor_tensor(t1k, pskp[0:R2, :], cosk_sb, op=ALU.mult)
            nc.vector.tensor_tensor(t2k, pskp[R2:QK_ROPE, :], sink_sb, op=ALU.mult)
            nc.vector.tensor_tensor(kpeT8[0:R2, :], t1k, t2k, op=ALU.subtract)
            t1k2 = apool.tile([R2, M], F32, tag="ropek1")
            t2k2 = apool.tile([R2, M], F32, tag="ropek2")
            nc.vector.tensor_tensor(t1k2, pskp[0:R2, :], sink_sb, op=ALU.mult)
            nc.vector.tensor_tensor(t2k2, pskp[R2:QK_ROPE, :], cosk_sb, op=ALU.mult)
            nc.vector.tensor_tensor(kpeT8[R2:QK_ROPE, :], t1k2, t2k2, op=ALU.add)

            # ---------- q path ----------
            # streamed weights force k-outer loops; PSUM cannot hold multiple
            # interleaved accumulation groups per bank, so each (k, chunk)
            # product is a closed single-matmul group, accumulated on DVE.
            def acc_chunk(acc_slice, phk, first):
                if first:
                    nc.vector.tensor_copy(acc_slice, phk)
                else:
                    nc.vector.tensor_tensor(acc_slice, acc_slice, phk, op=ALU.add)

            q1T_acc = apool.tile([128, KB, M], F32)
            for k in range(KQ):
                wqa_sb = wqa_pool.tile([128, KB, 128], BF16, tag="wqa")
                nc.scalar.dma_start(wqa_sb, wqa[k, :, :, :])
                for ng in range(KB // 4):
                    phk = ps_phk.tile([128, 4, M], F32, tag="phk")
                    for j in range(4):
                        nc.tensor.matmul(phk[:, j, :], wqa_sb[:, ng * 4 + j, :],
                                         xT_sb[:, k, :], start=True, stop=True)
                    acc_chunk(q1T_acc[:, ng * 4:(ng + 1) * 4, :], phk, k == 0)
            bcq = rms_factor(q1T_acc, KB, Q_LORA)
            q1nT = apool.tile([128, KB, M], BF16)
            for kc in range(KB):
                nc.vector.scalar_tensor_tensor(
                    out=q1nT[:, kc, :], in0=q1T_acc[:, kc, :],
                    scalar=qnwT_sb[:, kc, :], in1=bcq,
                    op0=ALU.mult, op1=ALU.mult)

            qnT_acc = apool.tile([128, N_HEADS, M], F32)
            # rope-dim accumulators split at base partition 0 (SB-SB tensor
            # ops require equal base partitions on both inputs)
            qpe_e = apool.tile([R2, N_HEADS, M], F32)
            qpe_o = apool.tile([R2, N_HEADS, M], F32)
            for k in range(KB):
                wqbn_sb = wqb_pool.tile([128, N_HEADS, 128], BF16, tag="wqbn")
                nc.sync.dma_start(wqbn_sb, wqbn[k, :, :, :])
                wqbp_sb = wqbp_pool.tile([128, N_HEADS, QK_ROPE], BF16, tag="wqbp")
                nc.sync.dma_start(wqbp_sb, wqbp[k, :, :, :])
                for hg in range(N_HEADS // 4):
                    phk = ps_phk.tile([128, 4, M], F32, tag="phk")
                    for j in range(4):
                        nc.tensor.matmul(phk[:, j, :],
                                         wqbn_sb[:, hg * 4 + j, :],
                                         q1nT[:, k, :], start=True, stop=True)
                    acc_chunk(qnT_acc[:, hg * 4:(hg + 1) * 4, :], phk, k == 0)
                for hg in range(N_HEADS // 4):
                    php = ps_phk.tile([QK_ROPE, 4, M], F32, tag="phk")
                    for j in range(4):
                        nc.tensor.matmul(php[:, j, :],
                                         wqbp_sb[:, hg * 4 + j, :],
                                         q1nT[:, k, :], start=True, stop=True)
                    acc_chunk(qpe_e[:, hg * 4:(hg + 1) * 4, :],
                              php[0:R2, :, :], k == 0)
                    acc_chunk(qpe_o[:, hg * 4:(hg + 1) * 4, :],
                              php[R2:QK_ROPE, :, :], k == 0)

            if debug:
                def dump(dram_ap, src_ap, shape):
                    t = apool.tile(shape, F32, tag="dbg")
                    nc.vector.tensor_copy(t, src_ap)
                    nc.sync.dma_start(dram_ap, t)
                dump(dq1nT[:, :, :], q1nT, [128, KB, M])
                dump(dkvlatT[:, :, :], kvlatT_bf, [128, 4, M])
                dump(dkpeT[:, :], kpeT8, [QK_ROPE, M])

            qnT_sb = apool.tile([128, N_HEADS, M], BF16)
            nc.vector.tensor_copy(qnT_sb, qnT_acc)

            # QT[:, 0:4, b, hs] = absorbed nope; QT[:64, 4, b, hs] = roped pe
            QT = apool.tile([128, 5, BPC, 64], BF16)

            # q_pe rope (all heads, even/odd-split rows)
            t1q = apool.tile([R2, N_HEADS, M], F32, tag="ropeq1")
            t2q = apool.tile([R2, N_HEADS, M], F32, tag="ropeq2")
            nc.vector.tensor_tensor(t1q, qpe_e, cosq_sb, op=ALU.mult)
            nc.vector.tensor_tensor(t2q, qpe_o, sinq_sb, op=ALU.mult)
            for b in range(BPC):
                nc.vector.tensor_tensor(
                    QT[0:R2, 4, b, :].rearrange("p (h s) -> p h s", h=N_HEADS),
                    t1q[:, :, b * SEQLEN:(b + 1) * SEQLEN],
                    t2q[:, :, b * SEQLEN:(b + 1) * SEQLEN],
                    op=ALU.subtract)
            t3q = apool.tile([R2, N_HEADS, M], F32, tag="ropeq1")
            t4q = apool.tile([R2, N_HEADS, M], F32, tag="ropeq2")
            nc.vector.tensor_tensor(t3q, qpe_e, sinq_sb, op=ALU.mult)
            nc.vector.tensor_tensor(t4q, qpe_o, cosq_sb, op=ALU.mult)
            for b in range(BPC):
                nc.vector.tensor_tensor(
                    QT[R2:QK_ROPE, 4, b, :].rearrange("p (h s) -> p h s", h=N_HEADS),
                    t3q[:, :, b * SEQLEN:(b + 1) * SEQLEN],
                    t4q[:, :, b * SEQLEN:(b + 1) * SEQLEN],
                    op=ALU.add)

            # absorb: QT[:, cc, b, h*4+s] = sum_d wkvbn[h][d, c] * qnope[m, h, d]
            for h in range(N_HEADS):
                pa4 = ps_small.tile([128, 4, M], F32, tag="ps")
                for cc in range(4):
                    nc.tensor.matmul(pa4[:, cc, :],
                                     wkvbn_sb[:, h, cc * 128:(cc + 1) * 128],
                                     qnT_sb[:, h, :], start=True, stop=True)
                for cc in range(4):
                    nc.vector.tensor_copy(
                        QT[:, cc, :, h * SEQLEN:(h + 1) * SEQLEN],
                        pa4[:, cc, :].rearrange("p (b s) -> p b s", b=BPC))

            if debug:
                dump(dQT[:, :, :, :], QT, [128, 5, BPC, 64])

            # ---------- attention ----------
            outT = apool.tile([128, 4, N_HEADS, M], BF16)
            for b in range(BPC):
                po = ps_pv.tile([128, 4, 64], F32, tag="po")
                ssum = ps_sum.tile([1, 512], F32, tag="ssum")
                PTs = []
                kvs = []
                for g in range(G8):
                    kl = kl_pool.tile([128, 4, NTG], E3, tag="kl")
                    nc.sync.dma_start(kl, klatS[b, g])
                    pe_t = pe_pool.tile([QK_ROPE, NTG], E3, tag="pe")
                    nc.sync.dma_start(pe_t, peS[b, g])
                    if g % 2 == 0:
                        kv16 = kv_pool.tile([128, 16, 512], E3, tag="kv16")
                        nc.scalar.dma_start(kv16, kvnP[b, g // 2])
                        kvs.append(kv16)
                    if g == G8 - 1:
                        # overwrite rows start_pos..start_pos+3 with fresh values
                        for j in range(4):
                            nc.sync.dma_start(
                                kl[:, j, NTG - SEQLEN:],
                                kvlatT8[:, j, b * SEQLEN:(b + 1) * SEQLEN])
                        nc.sync.dma_start(
                            pe_t[:, NTG - SEQLEN:],
                            kpeT8[:, b * SEQLEN:(b + 1) * SEQLEN])
                        nc.sync.dma_start(
                            kvs[-1][128 - SEQLEN:, 15, :],
                            kvlat8[b * SEQLEN:(b + 1) * SEQLEN, :])
                    sp = ps_sc.tile([128, 8, 64], F32, tag="sp")
                    for i in range(8):
                        for j in range(4):
                            nc.tensor.matmul(sp[:, i, :],
                                             kl[:, j, i * 128:(i + 1) * 128],
                                             QT[:, j, b, :],
                                             start=(j == 0), stop=False)
                        nc.tensor.matmul(sp[:, i, :],
                                         pe_t[:, i * 128:(i + 1) * 128],
                                         QT[0:QK_ROPE, 4, b, :],
                                         start=False, stop=True)
                    PT = pt_pool.tile([128, 8, 64], BF16, tag="PT")
                    nc.scalar.activation(out=PT, in_=sp, func=AF.Exp, scale=SCALE)
                    PTs.append(PT)
                    # row sums via ones-matmul, one PSUM group, lagged one
                    # g-iteration so PE never stalls on the exp
                    if g > 0:
                        nc.tensor.matmul(ssum, ones_cb,
                                         PTs[g - 1].rearrange("p a h -> p (a h)"),
                                         start=(g == 1), stop=False)
                nc.tensor.matmul(ssum, ones_cb,
                                 PTs[G8 - 1].rearrange("p a h -> p (a h)"),
                                 start=False, stop=True)

                # PV: cc-outer so each PSUM-bank accumulation group is
                # open exclusively (interleaved groups in one bank corrupt)
                for cc in range(4):
                    for g in range(G8):
                        for i in range(8):
                            nc.tensor.matmul(
                                po[:, cc, :],
                                kvs[g // 2][:, (g % 2) * 8 + i,
                                            cc * 128:(cc + 1) * 128],
                                PTs[g][:, i, :],
                                start=(g == 0 and i == 0),
                                stop=(g == G8 - 1 and i == 7))

                # 1/rowsum: tree-reduce the per-chunk partials [1,(i,h)]->[1,h]
                ssum_sb = apool.tile([1, 512], F32, tag="att_ssb")
                nc.vector.tensor_copy(ssum_sb, ssum)
                s4 = apool.tile([1, 256], F32, tag="att_r4")
                nc.vector.tensor_tensor(s4, ssum_sb[:, 0:256],
                                        ssum_sb[:, 256:512], op=ALU.add)
                s2 = apool.tile([1, 128], F32, tag="att_r2")
                nc.vector.tensor_tensor(s2, s4[:, 0:128], s4[:, 128:256],
                                        op=ALU.add)
                red = apool.tile([1, 64], F32, tag="att_red")
                nc.vector.tensor_tensor(red, s2[:, 0:64], s2[:, 64:128],
                                        op=ALU.add)
                rinv = apool.tile([1, 64], F32, tag="att_rinv")
                nc.vector.reciprocal(rinv, red)
                if debug:
                    dump(dssum[b], ssum_sb, [1, 512])
                    dump(dred[b], red, [1, 64])
                bc_ps = ps_small.tile([128, 64], F32, tag="ps")
                nc.tensor.matmul(bc_ps, ones_rf, rinv, start=True, stop=True)
                bc = apool.tile([128, 64], F32, tag="att_bc")
                nc.vector.tensor_copy(bc, bc_ps)
                for cc in range(4):
                    nc.vector.tensor_tensor(
                        outT[:, cc, :, b * SEQLEN:(b + 1) * SEQLEN],
                        po[:, cc, :].rearrange("p (h s) -> p h s", h=N_HEADS),
                        bc.rearrange("p (h s) -> p h s", h=N_HEADS),
                        op=ALU.mult)

            if debug:
                dump(doutT[:, :, :, :], outT, [128, 4, N_HEADS, M])

            # ---------- v-proj: o2T[d, h, m] ----------
            o2T = apool.tile([128, N_HEADS, M], BF16)
            for h in range(N_HEADS):
                pv = ps_small.tile([128, M], F32, tag="ps")
                for cc in range(4):
                    nc.tensor.matmul(pv, wkvbv_sb[:, h, cc, :],
                                     outT[:, cc, h, :],
                                     start=(cc == 0), stop=(cc == 3))
                nc.vector.tensor_copy(o2T[:, h, :], pv)

            if debug:
                dump(do2T[:, :, :], o2T, [128, N_HEADS, M])

            # ---------- final: out = o2 @ wo ----------
            for n in range(4):
                pf = ps_small.tile([M, 512], F32, tag="ps")
                for kk in range(4):
                    wot = wo_pool.tile([128, 4, 512], BF16, tag="wo")
                    nc.gpsimd.dma_start(wot, wo[n, kk])
                    for j in range(4):
                        k = kk * 4 + j
                        nc.tensor.matmul(pf, o2T[:, k, :], wot[:, j, :],
                                         start=(k == 0), stop=(k == KQ - 1))
                fin = pt_pool.tile([M, 512], F32, tag="fin")
                nc.vector.tensor_copy(fin, pf)
                nc.sync.dma_start(out[:, n * 512:(n + 1) * 512], fin)

    nc.compile()
    return nc


_NC_CACHE = {}

# even/odd split permutation for rope dims: rows 0..31 = even pairs, 32..63 = odd
_PERM = np.concatenate([np.arange(0, QK_ROPE, 2), np.arange(1, QK_ROPE, 2)])


def prep_in_maps(x, wq_a, q_norm_w, wq_b, wkv_a, kv_norm_w, wkv_b, wo,
                 kv_cache, pe_cache, freqs_cos, freqs_sin, start_pos):
    assert int(start_pos) == START_POS
    bf = lambda a: np.ascontiguousarray(np.asarray(a, dtype=np.float32), dtype=NBF)
    f32 = lambda a: np.ascontiguousarray(np.asarray(a), dtype=np.float32)
    c = np.ascontiguousarray

    x = f32(x)
    wq_a = f32(wq_a); wq_b = f32(wq_b); wkv_a = f32(wkv_a)
    wkv_b_r = f32(wkv_b).reshape(N_HEADS, QK_NOPE + V_DIM, KV_LORA)
    wo_f = f32(wo)

    # --- weights, transposed/tiled layouts (bf16) ---
    wqa_t = bf(wq_a.reshape(KQ, 128, KB, 128))                    # [k,p,nc,d]
    wqb_r = wq_b.reshape(KB, 128, N_HEADS, QK_HD)                 # [k,p,h,d]
    wqbn_t = bf(wqb_r[:, :, :, :QK_NOPE])
    wqbp_t = bf(wqb_r[:, :, :, QK_NOPE:][:, :, :, _PERM])         # rope perm
    wkva_r = wkv_a.reshape(KQ, 128, KV_LORA + QK_ROPE)
    wkval_t = bf(wkva_r[:, :, :KV_LORA].reshape(KQ, 128, 4, 128)
                 .transpose(1, 0, 2, 3))                          # [p,k,j,d]
    wkvap_t = bf(wkva_r[:, :, KV_LORA:][:, :, _PERM].transpose(1, 0, 2))
    wkvbn_t = bf(wkv_b_r[:, :QK_NOPE, :].transpose(1, 0, 2))      # [d,h,c]
    # [c-in-chunk, h, cc, d]
    wkvbv_t = bf(wkv_b_r[:, QK_NOPE:, :].transpose(2, 0, 1)
                 .reshape(4, 128, N_HEADS, V_DIM).transpose(1, 2, 0, 3))
    # [n, kk, p, j, cc]: wo_t[n,kk,p,j,cc] = wo[(kk*4+j)*128+p, n*512+cc]
    wo_t = bf(wo_f.reshape(4, 4, 128, 4, 512).transpose(3, 0, 2, 1, 4))

    qnwT = c(f32(q_norm_w).reshape(KB, 128, 1).transpose(1, 0, 2))
    kvnwT = c(f32(kv_norm_w).reshape(4, 128, 1).transpose(1, 0, 2))

    # --- rope tables, transposed to [pair, (h), m], even/odd split is implicit
    # (row r < 32 uses cos[r], row 32+r also uses cos[r]; table holds cos[i,m])
    cos = f32(freqs_cos); sin = f32(freqs_sin)                    # [s=4, 32]
    cosT = np.tile(cos.T, (1, BPC))                               # [32, 16] m=(b,s)
    sinT = np.tile(sin.T, (1, BPC))
    cosq_t = c(np.repeat(cosT[:, None, :], N_HEADS, axis=1))      # [32, h, 16]
    sinq_t = c(np.repeat(sinT[:, None, :], N_HEADS, axis=1))

    # --- fp8 caches in both layouts ---
    kv8 = np.asarray(kv_cache, dtype=np.float32).astype(NE3)      # [32, 8192, 512]
    pe8 = np.asarray(pe_cache, dtype=np.float32).astype(NE3)      # [32, 8192, 64]

    in_maps = []
    for ci in range(N_CORES):
        bs = slice(ci * BPC, (ci + 1) * BPC)
        kvb = kv8[bs]                                             # [4, 8192, 512]
        peb = pe8[bs]
        # scores-side: klatS[b,g,p,j,n] = kv[b, g*1024+n, j*128+p]
        klatS = c(kvb.reshape(BPC, G8, NTG, 4, 128).transpose(0, 1, 4, 3, 2))
        # peS[b,g,r,n] = pe[b, g*1024+n, perm(r)]
        peS_ = c(peb[:, :, _PERM].reshape(BPC, G8, NTG, QK_ROPE)
                 .transpose(0, 1, 3, 2))
        # pv-side: kvnP[b,gg,p,i,c] = kv[b, gg*2048 + i*128 + p, c]
        kvnP_ = c(kvb.reshape(BPC, 4, 16, 128, KV_LORA).transpose(0, 1, 3, 2, 4))
        xc = bf(x[bs].reshape(M, DIM).T.reshape(KQ, 128, M).transpose(1, 0, 2))
        in_maps.append({
            "xT": xc,
            "wqa": wqa_t, "wqbn": wqbn_t, "wqbp": wqbp_t,
            "wkval": wkval_t, "wkvap": wkvap_t,
            "wkvbn": wkvbn_t, "wkvbv": wkvbv_t, "wo": wo_t,
            "qnwT": qnwT, "kvnwT": kvnwT,
            "cosq": cosq_t, "sinq": sinq_t, "cosk": c(cosT), "sink": c(sinT),
            "klatS": klatS, "peS": peS_, "kvnP": kvnP_,
        })
    return in_maps


def kernel(**inputs):
    in_maps = prep_in_maps(**inputs)

    debug = os.environ.get("KERNEL_DEBUG", "0") == "1"
    if "nc" not in _NC_CACHE:
        _NC_CACHE["nc"] = build_bass(debug=debug)
    nc = _NC_CACHE["nc"]

    trace = os.environ.get("KERNEL_TRACE", "0") == "1"
    res = run_bass_kernel_spmd(nc, in_maps, core_ids=list(range(N_CORES)), trace=trace)
    if trace and res.exec_time_ns is not None:
        print(f"HW exec time: {res.exec_time_ns} ns")
        _NC_CACHE["last_exec_ns"] = res.exec_time_ns

    _NC_CACHE["results"] = res.results
    outs = [r["out"].reshape(BPC, SEQLEN, DIM) for r in res.results]
    return np.concatenate(outs, axis=0).astype(np.float32)
